# revision 24
# baseline (speedup 1.0000x reference)
"""NeuralSuperSampling Trainium2 kernel (Bass/Tile, 8 NeuronCores) — v2.

Only the reweighting-net path affects the output: rgbd = [zero-upsampled f0
rgb+depth, accumulatively-warped color+depth of frames 1..4], then 3 convs +
tanh. Feature-extraction nets and the YCbCr conversion are dead code w.r.t.
the output and skipped.

Sharding: 8 cores = 2 batches x 4 bands of 128 output rows (at 512x896).

Warp (exact bilinear, zero padding) via per-offset hat masks:
  out = sum_{dy,kx} hat(my-dy)*hat(mx-kx)*src[y+dy,x+kx]
with (dy,kx) pairs pruned to the exact 2D support of (floor(my),floor(mx))
computed on the host from a bit-exact replica of the device motion upsample.

v2 structure:
- masks built on the ACT engine (Abs + Relu with per-partition bias APs).
- staging tensors carry pre-zeroed x-margins -> no per-dy memsets.
- each frame's FIRST warp exploits the zero-stuffed source: only odd
  rows/cols are populated, so parity subchunks with a compact (half-width)
  source and a duplicate-expansion tile cut that warp to ~1/4 cost.
- remainder rows are packed (channel,row)->partitions to keep ops full.
- a fraction of dy-chains runs on the Pool (gpsimd) engine in parallel.
"""

import numpy as np

S = 2
I = 5
B, H, W = 2, 256, 448
Hn, Wn = H * S, W * S
NCORES = 8
NBAND = 4
BAND = Hn // NBAND  # 128
NR = 256            # staging rows per core (absolute row index space)
NRC = 128           # compact staging rows
MR = 132            # motion shard rows (orig res, clamp-padded by 1)
MW = 450            # motion shard cols (clamp-padded by 1 each side)
CM = 3              # conv margin rows
RCH = 32            # conv row-chunk
POOL_FRAC = 0.18    # fraction of dense dy-chains offloaded to gpsimd


# ---------------------------------------------------------------------------
# host-side exact motion upsample (bit-identical to the device arithmetic)
# ---------------------------------------------------------------------------

def _upsample_exact(m):
    """[H,W] f32 -> [Hn,Wn] f32, matching the device op order exactly."""
    f = np.float32
    m = m.astype(f)
    pad = np.pad(m, [(1, 1), (1, 1)], mode="edge")
    p0, p1, p2 = pad[0:-2, :], pad[1:-1, :], pad[2:, :]
    ye = (p1 * f(0.75)) + (p0 * f(0.25))
    yo = (p1 * f(0.75)) + (p2 * f(0.25))
    rows = np.zeros((Hn, W + 2), f)
    rows[0::2] = ye
    rows[1::2] = yo
    y0, y1, y2 = rows[:, 0:-2], rows[:, 1:-1], rows[:, 2:]
    xe = (y1 * f(0.75)) + (y0 * f(0.25))
    xo = (y1 * f(0.75)) + (y2 * f(0.25))
    res = np.empty((Hn, Wn), f)
    res[:, 0::2] = xe
    res[:, 1::2] = xo
    return res


def _plan(motion):
    """JIT plan: windows, spans, chunking and exact (dy,kx) support lists."""
    motion = np.asarray(motion, np.float32)
    # exact upsampled motion fields per (batch, stage): [B,4,2,Hn,Wn]
    mv = np.empty((B, 4, 2, Hn, Wn), np.float32)
    for b in range(B):
        for j in range(4):
            mv[b, j, 0] = _upsample_exact(motion[b, j, 0])
            mv[b, j, 1] = _upsample_exact(motion[b, j, 1])

    Ly, Hy, Lx, Hx = [], [], [], []
    for j in range(4):
        Ly.append(int(np.floor(mv[:, j, 1].min())))
        Hy.append(int(np.floor(mv[:, j, 1].max())) + 1)
        Lx.append(int(np.floor(mv[:, j, 0].min())))
        Hx.append(int(np.floor(mv[:, j, 0].max())) + 1)

    cumLo = [0] * 5
    cumHi = [0] * 5
    for j in range(4):
        cumLo[j + 1] = cumLo[j] + Ly[j]
        cumHi[j + 1] = cumHi[j] + Hy[j]
    rl = cumLo[4] - CM - 1
    RLoff = rl - (rl & 1)
    span_last = (BAND + cumHi[4] + CM) - RLoff
    assert span_last <= NR, f"staging rows {span_last} > {NR}"

    s0 = -RLoff
    out_lo = [s0 + cumLo[j] - CM for j in range(5)]
    out_hi = [s0 + BAND + cumHi[j] + CM for j in range(5)]
    assert out_lo[4] >= 0 and out_hi[4] <= NR

    # image row for staging row r on core (b,g): RL + r, RL = g*BAND + RLoff
    RLs = [g * BAND + RLoff for g in range(NBAND)]

    def stage_rows_pixels(j, r0, rows, parity=None):
        """Collect (my, mx) values over all cores for staging rows
        [r0, r0+rows) (optionally only rows of given parity), stage j."""
        mys, mxs = [], []
        rr = np.arange(r0, r0 + rows)
        if parity is not None:
            rr = rr[(rr % 2) == parity]
        for g in range(NBAND):
            arows = RLs[g] + rr
            sel = (arows >= 0) & (arows < Hn)
            if not sel.any():
                continue
            a = arows[sel]
            for b in range(B):
                mys.append(mv[b, j, 1][a, :])
                mxs.append(mv[b, j, 0][a, :])
        my = np.concatenate([x.ravel() for x in mys])
        mx = np.concatenate([x.ravel() for x in mxs])
        return my, mx

    def dense_support(j, r0, rows):
        """Exact sorted list of (dy, [kx...]) with nonzero hat-product."""
        my, mx = stage_rows_pixels(j, r0, rows)
        iy = np.floor(my).astype(np.int64)
        ix = np.floor(mx).astype(np.int64)
        fy = my - iy
        fx = mx - ix
        cells = set()
        pairs = [(iy, ix, None), (iy, ix + 1, fx > 0), (iy + 1, ix, fy > 0),
                 (iy + 1, ix + 1, (fx > 0) & (fy > 0))]
        for a, bb, m in pairs:
            if m is None:
                z = (a + 500) * 2000 + (bb + 500)
            else:
                z = (a[m] + 500) * 2000 + (bb[m] + 500)
            for v in np.unique(z):
                cells.add((int(v) // 2000 - 500, int(v) % 2000 - 500))
        bydy = {}
        for dy, kx in cells:
            bydy.setdefault(dy, []).append(kx)
        return [(dy, sorted(kxs)) for dy, kxs in sorted(bydy.items())]

    def sparse_support(j, r0, rows, parity, KLO, KC):
        """(dy, [kc...]) support for the compact first-warp, given output-row
        parity (staging-row parity == image-row parity; RL even)."""
        my, mx = stage_rows_pixels(j, r0, rows, parity=parity)
        # PAR = 1 for even output col, 0 for odd
        par = np.tile((1 - (np.arange(Wn) % 2)).astype(np.float32),
                      my.size // Wn)
        mxadj = mx - par
        dy_par = 1 - parity  # source staging row must be odd
        iy = np.floor(my).astype(np.int64)
        fy = my - iy
        k1 = np.floor((mxadj + 1.0) / 2.0).astype(np.int64)  # candidate tap
        kok = np.abs(mxadj - 2.0 * k1) < 1.0
        cells = set()
        for dy_arr, m in [(iy, kok), (iy + 1, kok & (fy > 0))]:
            dsel = ((dy_arr % 2) + 2) % 2 == dy_par
            mm = m & dsel
            if not mm.any():
                continue
            z = (dy_arr[mm] + 500) * 2000 + (k1[mm] - KLO + 500)
            for v in np.unique(z):
                dy = int(v) // 2000 - 500
                kc = int(v) % 2000 - 500
                if 0 <= kc < KC:
                    cells.add((dy, kc))
        bydy = {}
        for dy, kc in cells:
            bydy.setdefault(dy, []).append(kc)
        return [(dy, sorted(ks)) for dy, ks in sorted(bydy.items())]

    stages = []
    for j in range(4):
        nch_d = 4 * (3 - j)          # dense channels (frames j+2..4)
        span0, span1 = out_lo[j], out_hi[j]
        span = span1 - span0
        KLO = (Lx[j] - 1) // 2
        KHI = (Hx[j] + 1 - 1) // 2
        KC = KHI - KLO + 1
        dense_chunks = []
        if nch_d > 0:
            r = span0
            while r < span1:
                rows = min(128, span1 - r)
                sup = dense_support(j, r, rows)
                if rows == 128:
                    sets = []
                    c0 = 0
                    while c0 < nch_d:
                        gg = min(8, nch_d - c0)
                        sets.append(("nat", c0, gg))
                        c0 += gg
                else:
                    g = min(nch_d, 128 // rows)
                    sets = []
                    c0 = 0
                    while c0 < nch_d:
                        gg = min(g, nch_d - c0)
                        sets.append(("pack", c0, gg))
                        c0 += gg
                dense_chunks.append(dict(r0=r, rows=rows, sup=sup, sets=sets))
                r += rows
        sparse_subs = []
        for parity in (0, 1):
            rr = np.arange(span0, span1)
            rr = rr[(rr % 2) == parity]
            r0p, R2 = int(rr[0]), len(rr)
            assert R2 <= 128
            sup = sparse_support(j, span0, span, parity, KLO, KC)
            sparse_subs.append(dict(parity=parity, r0=r0p, rows=R2, sup=sup))
        stages.append(dict(j=j, nch_d=nch_d, KLO=KLO, KC=KC,
                           dense=dense_chunks, sparse=sparse_subs))

    plan = dict(Ly=Ly, Hy=Hy, Lx=Lx, Hx=Hx, cumLo=cumLo, cumHi=cumHi,
                RLoff=RLoff, out_lo=out_lo, out_hi=out_hi, stages=stages)

    # bias-constant table: one column per needed f32 value
    vals = {1.0}
    for st in stages:
        for ch in st["dense"]:
            for dy, kxs in ch["sup"]:
                vals.add(float(-dy))
                for kx in kxs:
                    vals.add(float(-kx))
        for sub in st["sparse"]:
            for dy, kcs in sub["sup"]:
                vals.add(float(-dy))
                for kc in kcs:
                    vals.add(float(-2 * (kc + st["KLO"])))
    bias_vals = sorted(vals)
    plan["bias_vals"] = bias_vals
    plan["bias_idx"] = {v: i for i, v in enumerate(bias_vals)}
    return plan


def _plan_key(plan):
    def sup_key(sup):
        return tuple((dy, tuple(k)) for dy, k in sup)
    parts = [tuple(plan["Ly"]), tuple(plan["Hy"]), tuple(plan["Lx"]),
             tuple(plan["Hx"]), plan["RLoff"]]
    for st in plan["stages"]:
        for ch in st["dense"]:
            parts.append((ch["r0"], ch["rows"], sup_key(ch["sup"]),
                          tuple(ch["sets"])))
        for sub in st["sparse"]:
            parts.append((sub["parity"], sub["r0"], sub["rows"],
                          sup_key(sub["sup"])))
    return tuple(map(str, parts))


# ---------------------------------------------------------------------------
# device kernel build
# ---------------------------------------------------------------------------

def _build(plan):
    import concourse.bacc as bacc
    import concourse.mybir as mybir
    from concourse import tile

    f32 = mybir.dt.float32
    bf16 = mybir.dt.bfloat16
    AOP = mybir.AluOpType
    ACT = mybir.ActivationFunctionType

    Ly, Hy, Lx, Hx = plan["Ly"], plan["Hy"], plan["Lx"], plan["Hx"]
    RLoff = plan["RLoff"]
    out_lo, out_hi = plan["out_lo"], plan["out_hi"]
    bias_vals = plan["bias_vals"]
    bidx = plan["bias_idx"]
    NBV = len(bias_vals)
    s0 = -RLoff

    # st[j] margins are set by their reader: stage j-1 (window of mv[j-1]),
    # st[0] is read by the conv (margin CM).
    stL = [0] * 4
    stR = [0] * 4
    stL[0], stR[0] = CM, CM
    for j in range(1, 4):
        stL[j], stR[j] = -Lx[j - 1], Hx[j - 1] + 1
    stXW = [Wn + stL[j] + stR[j] for j in range(4)]
    nch_st = [4 * (4 - j) for j in range(4)]

    nc = bacc.Bacc("TRN2", target_bir_lowering=False, debug=False)

    # ---- external inputs ----
    zu0 = nc.dram_tensor("zu0", [4, NR, Wn + 2 * CM], bf16,
                         kind="ExternalInput").ap()
    zcs = [nc.dram_tensor(f"zc{f}", [4, NRC, W - plan["stages"][f - 1]["KLO"]
                                     + plan["stages"][f - 1]["KC"] + 2],
                          bf16, kind="ExternalInput").ap()
           for f in range(1, 5)]
    mot = nc.dram_tensor("mot", [4, 2, MR, MW], f32, kind="ExternalInput").ap()
    rv = nc.dram_tensor("rv", [NR, 1], f32, kind="ExternalInput").ap()
    dyc = nc.dram_tensor("dyc", [128, NBV], f32, kind="ExternalInput").ap()
    par = nc.dram_tensor("par", [1, Wn], f32, kind="ExternalInput").ap()
    w1 = nc.dram_tensor("w1", [3, 60, 32], bf16, kind="ExternalInput").ap()
    w2 = nc.dram_tensor("w2", [3, 96, 32], bf16, kind="ExternalInput").ap()
    w3 = nc.dram_tensor("w3", [3, 96, 4], bf16, kind="ExternalInput").ap()
    b1 = nc.dram_tensor("b1", [32, 1], f32, kind="ExternalInput").ap()
    b2 = nc.dram_tensor("b2", [32, 1], f32, kind="ExternalInput").ap()
    b3 = nc.dram_tensor("b3", [4, 1], f32, kind="ExternalInput").ap()
    out = nc.dram_tensor("out", [4, BAND, Wn], f32, kind="ExternalOutput").ap()

    mvd = nc.dram_tensor("mvd", [4, 2, NR, Wn], f32).ap()
    st = [nc.dram_tensor(f"st{j}", [nch_st[j], NR, stXW[j]], bf16).ap()
          for j in range(4)]

    with tile.TileContext(nc) as tc:
        # ---------------- resident tiles ----------------
        with tc.tile_pool(name="resident", bufs=1) as rpool:
            dyct = rpool.tile([128, NBV], f32, tag="dyct")
            nc.sync.dma_start(dyct[:], dyc[:, :])
            part = rpool.tile([128, Wn], f32, tag="part")
            nc.gpsimd.dma_start(
                part[:], par[0:1, :].broadcast_to((128, Wn)))
            zt = rpool.tile([128, 64], bf16, tag="zt")
            nc.vector.memset(zt[:], 0.0)

            def bias_ap(v, P):
                return dyct[0:P, bidx[float(v)]:bidx[float(v)] + 1]

            # ---------------- zero st margins (one-time) ----------------
            for j in range(4):
                for (x0, wdt) in ((0, stL[j]), (stL[j] + Wn, stR[j])):
                    if wdt == 0:
                        continue
                    for c in range(nch_st[j]):
                        nc.sync.dma_start(
                            st[j][c, :, x0:x0 + wdt]
                            .rearrange("(rr r) w -> rr r w", r=2),
                            zt[0:128, 0:2 * wdt]
                            .rearrange("p (r w) -> p r w", r=2))

            # ---------------- motion upsample (exact jax bilinear 2x) ------
            with tc.tile_pool(name="mvp", bufs=1) as pool:
                FREE = 4 * 2 * MW
                for u0 in (0, NR // 2):
                    o0 = u0 // 2
                    p0 = pool.tile([64, FREE], f32, tag="p0")
                    p1 = pool.tile([64, FREE], f32, tag="p1")
                    p2 = pool.tile([64, FREE], f32, tag="p2")
                    for t_, off in ((p0, 0), (p1, 1), (p2, 2)):
                        nc.sync.dma_start(
                            t_[:].rearrange("p (a b w) -> p a b w", a=4, b=2),
                            mot[:, :, o0 + off:o0 + off + 64, :]
                            .rearrange("a b r w -> r a b w"))
                    ye = pool.tile([64, FREE], f32, tag="ye")
                    yo = pool.tile([64, FREE], f32, tag="yo")
                    t0 = pool.tile([64, FREE], f32, tag="t0")
                    nc.vector.tensor_scalar_mul(t0[:], p0[:], 0.25)
                    nc.vector.scalar_tensor_tensor(ye[:], p1[:], 0.75, t0[:],
                                                   op0=AOP.mult, op1=AOP.add)
                    nc.vector.tensor_scalar_mul(t0[:], p2[:], 0.25)
                    nc.vector.scalar_tensor_tensor(yo[:], p1[:], 0.75, t0[:],
                                                   op0=AOP.mult, op1=AOP.add)
                    ph = pool.tile([64, 4 * 2 * W], f32, tag="ph")
                    ph4 = ph[:].rearrange("p (a b w) -> p a b w", a=4, b=2)
                    for py, ysrc in ((0, ye), (1, yo)):
                        y4 = ysrc[:].rearrange("p (a b w) -> p a b w",
                                               a=4, b=2)
                        for px in range(2):
                            if px == 0:
                                nc.vector.tensor_scalar_mul(
                                    ph4, y4[:, :, :, 0:W], 0.25)
                            else:
                                nc.vector.tensor_scalar_mul(
                                    ph4, y4[:, :, :, 2:W + 2], 0.25)
                            nc.vector.scalar_tensor_tensor(
                                ph4, y4[:, :, :, 1:W + 1], 0.75, ph4,
                                op0=AOP.mult, op1=AOP.add)
                            for ai in range(4):
                                for bi in range(2):
                                    nc.sync.dma_start(
                                        mvd[ai, bi, u0 + py:u0 + 128:2,
                                            px:Wn:2],
                                        ph4[:, ai, bi, :])

            # ---------------- warp stages ----------------
            for stg in reversed(plan["stages"]):
                j = stg["j"]
                nch_d = stg["nch_d"]
                KLO, KC = stg["KLO"], stg["KC"]
                srcst = st[j + 1] if j < 3 else None
                mL_src = stL[j + 1] if j < 3 else 0
                XW = stXW[j + 1] if j < 3 else 0

                # ======== dense part: frames j+2..4 from st[j+1] ========
                for ci, ch in enumerate(stg["dense"]):
                    r0, rows, sup = ch["r0"], ch["rows"], ch["sup"]
                    dy_all = [d for d, _ in sup]
                    npool = (0 if len(dy_all) < 5 else
                             max(1, int(round(POOL_FRAC * len(dy_all)))))
                    pool_dys = set(dy_all[1::max(1, len(dy_all) // npool)]
                                   [:npool]) if npool else set()
                    sets = ch["sets"]
                    natural = sets[0][0] == "nat"
                    Gmax = max(g for _, _, g in sets)
                    PM = rows if natural else rows * Gmax  # mask partitions
                    tag = f"d{j}_{ci}"
                    with tc.tile_pool(name=f"wm_{tag}", bufs=1) as mp, \
                         tc.tile_pool(name=f"ws_{tag}", bufs=3) as sp:
                        mvx = mp.tile([PM, Wn], f32, tag="mvx")
                        mvy = mp.tile([PM, Wn], f32, tag="mvy")
                        rvt = mp.tile([PM, 1], f32, tag="rvt")
                        if natural:
                            nc.sync.dma_start(mvx[:],
                                              mvd[j, 0, r0:r0 + rows, :])
                            nc.sync.dma_start(mvy[:],
                                              mvd[j, 1, r0:r0 + rows, :])
                            nc.sync.dma_start(rvt[:], rv[r0:r0 + rows, :])
                        else:
                            for g in range(Gmax):
                                nc.sync.dma_start(
                                    mvx[g * rows:(g + 1) * rows, :],
                                    mvd[j, 0, r0:r0 + rows, :])
                                nc.sync.dma_start(
                                    mvy[g * rows:(g + 1) * rows, :],
                                    mvd[j, 1, r0:r0 + rows, :])
                                nc.sync.dma_start(
                                    rvt[g * rows:(g + 1) * rows, :],
                                    rv[r0:r0 + rows, :])
                        ut = mp.tile([PM, Wn], f32, tag="ut")
                        uy = mp.tile([PM, Wn], f32, tag="uy")
                        kx_all = sorted({k for _, ks in sup for k in ks})
                        mxk = {}
                        for kx in kx_all:
                            m = mp.tile([PM, Wn], bf16, tag=f"mx{kx}",
                                        name=f"mx{tag}_{kx}")
                            nc.scalar.activation(
                                ut[:], mvx[:], ACT.Abs,
                                bias=bias_ap(-kx, PM), scale=1.0)
                            nc.scalar.activation(
                                m[:], ut[:], ACT.Relu,
                                bias=bias_ap(1.0, PM), scale=-1.0)
                            mxk[kx] = m
                        # y-masks shared across sets: resident, one per dy
                        myk = {}
                        for dy in dy_all:
                            my = mp.tile([PM, Wn], bf16, tag=f"my{dy}",
                                         name=f"my{tag}_{dy}")
                            nc.scalar.activation(
                                uy[:], mvy[:], ACT.Abs,
                                bias=bias_ap(-dy, PM), scale=1.0)
                            nc.scalar.activation(
                                my[:], uy[:], ACT.Relu,
                                bias=rvt[:, 0:1], scale=-1.0)
                            myk[dy] = my
                        for (kind, c0, G) in sets:
                            stag = f"{tag}_{c0}"
                            if kind == "nat":
                                P, CH = rows, G
                                sbshape = [P, CH, XW]
                                tshape = [P, CH, Wn]
                            else:
                                P, CH = rows * G, 1
                                sbshape = [P, XW]
                                tshape = [P, Wn]
                            ap_cm = tc.tile_pool(name=f"wa_{stag}", bufs=1)
                            ap = ap_cm.__enter__()
                            acc = ap.tile(tshape, bf16, tag="acc",
                                          name=f"acc{stag}")
                            tmp = ap.tile(tshape, bf16, tag="tmp",
                                          name=f"tmp{stag}")
                            tm2 = ap.tile(tshape, bf16, tag="tm2",
                                          name=f"tm2{stag}")
                            accp = tmpp = tm2p = None
                            if pool_dys:
                                accp = ap.tile(tshape, bf16, tag="acp",
                                               name=f"accp{stag}")
                                tmpp = ap.tile(tshape, bf16, tag="tmq",
                                               name=f"tmpp{stag}")
                                tm2p = ap.tile(tshape, bf16, tag="tm2q",
                                               name=f"tm2p{stag}")
                            first_v = True
                            first_p = True
                            for (dy, kxs) in sup:
                                on_pool = dy in pool_dys
                                eng = nc.gpsimd if on_pool else nc.vector
                                sb = sp.tile(sbshape, bf16, tag="sb",
                                             name=f"sb{stag}_{dy}")
                                if kind == "nat":
                                    nc.sync.dma_start(
                                        sb[:],
                                        srcst[c0:c0 + CH,
                                              r0 + dy:r0 + dy + P, :]
                                        .rearrange("c r w -> r c w"))
                                else:
                                    for c2 in range(G):
                                        nc.sync.dma_start(
                                            sb[c2 * rows:(c2 + 1) * rows, :],
                                            srcst[c0 + c2,
                                                  r0 + dy:r0 + dy + rows, :])
                                t = tmpp if on_pool else tmp
                                a = accp if on_pool else acc
                                if kind == "nat":
                                    tv, av, t2 = t[:], a[:], tm2[:]
                                    myb = (myk[dy][:].unsqueeze(1)
                                           .broadcast_to((P, CH, Wn)))
                                    mkb = [(kx, mxk[kx][:].unsqueeze(1)
                                            .broadcast_to((P, CH, Wn)))
                                           for kx in kxs]
                                    srcs = [sb[:, :, kx - Lx[j]:
                                               kx - Lx[j] + Wn]
                                            for kx in kxs]
                                else:
                                    tv, av, t2 = t[:], a[:], tm2[:]
                                    myb = myk[dy][0:P, :]
                                    mkb = [(kx, mxk[kx][0:P, :])
                                           for kx in kxs]
                                    srcs = [sb[:, kx - Lx[j]:
                                               kx - Lx[j] + Wn]
                                            for kx in kxs]
                                for i in range(len(kxs)):
                                    if i == 0:
                                        eng.tensor_mul(tv, srcs[i],
                                                       mkb[i][1])
                                    else:
                                        eng.tensor_mul(t2, srcs[i],
                                                       mkb[i][1])
                                        eng.tensor_add(tv, tv, t2)
                                fst = first_p if on_pool else first_v
                                if fst:
                                    eng.tensor_mul(av, tv, myb)
                                    if on_pool:
                                        first_p = False
                                    else:
                                        first_v = False
                                else:
                                    eng.tensor_mul(t2, tv, myb)
                                    eng.tensor_add(av, av, t2)
                            if not first_p:
                                nc.vector.tensor_add(acc[:], acc[:],
                                                     accp[:])
                            if kind == "nat":
                                nc.sync.dma_start(
                                    st[j][4 + c0:4 + c0 + CH, r0:r0 + P,
                                          stL[j]:stL[j] + Wn]
                                    .rearrange("c r w -> r c w"), acc[:])
                            else:
                                for c2 in range(G):
                                    nc.sync.dma_start(
                                        st[j][4 + c0 + c2, r0:r0 + rows,
                                              stL[j]:stL[j] + Wn],
                                        acc[c2 * rows:(c2 + 1) * rows, :])
                            ap_cm.__exit__(None, None, None)

                # ======== sparse part: frame j+1 first warp from zc ========
                zc = zcs[j]  # frame j+1
                WCp = W - KLO + KC + 2
                WE2 = W + KC
                for sub in stg["sparse"]:
                    parity, r0p, R2 = sub["parity"], sub["r0"], sub["rows"]
                    sup = sub["sup"]
                    if not sup:
                        continue
                    tag = f"s{j}_{parity}"
                    with tc.tile_pool(name=f"sm_{tag}", bufs=1) as mp, \
                         tc.tile_pool(name=f"sy_{tag}", bufs=3) as myp, \
                         tc.tile_pool(name=f"se_{tag}", bufs=2) as ep:
                        mvx = mp.tile([R2, Wn], f32, tag="mvx")
                        mvy = mp.tile([R2, Wn], f32, tag="mvy")
                        nc.sync.dma_start(
                            mvx[:], mvd[j, 0, r0p:r0p + 2 * R2:2, :])
                        nc.sync.dma_start(
                            mvy[:], mvd[j, 1, r0p:r0p + 2 * R2:2, :])
                        rvt = mp.tile([R2, 1], f32, tag="rvt")
                        nc.sync.dma_start(rvt[:], rv[r0p:r0p + 2 * R2:2, :])
                        mxa = mp.tile([R2, Wn], f32, tag="mxa")
                        nc.vector.tensor_sub(mxa[:], mvx[:], part[0:R2, :])
                        ut = mp.tile([R2, Wn], f32, tag="ut")
                        kc_all = sorted({k for _, ks in sup for k in ks})
                        mxk = {}
                        for kc in kc_all:
                            m = mp.tile([R2, Wn], bf16, tag=f"mx{kc}",
                                        name=f"mx{tag}_{kc}")
                            nc.scalar.activation(
                                ut[:], mxa[:], ACT.Abs,
                                bias=bias_ap(-2 * (kc + KLO), R2), scale=1.0)
                            nc.scalar.activation(
                                m[:], ut[:], ACT.Relu,
                                bias=bias_ap(1.0, R2), scale=-1.0)
                            mxk[kc] = m
                        uy = mp.tile([R2, Wn], f32, tag="uy")
                        acc = mp.tile([R2, 4, Wn], bf16, tag="acc")
                        tmp = mp.tile([R2, 4, Wn], bf16, tag="tmp")
                        tm2 = mp.tile([R2, 4, Wn], bf16, tag="tm2")
                        first = True
                        for (dy, kcs) in sup:
                            rcb = (r0p + dy - 1) // 2
                            C = ep.tile([R2, 4, WCp], bf16, tag="C")
                            nc.sync.dma_start(
                                C[:], zc[:, rcb:rcb + R2, :]
                                .rearrange("c r w -> r c w"))
                            E = ep.tile([R2, 4, WE2, 2], bf16, tag="E")
                            nc.scalar.activation(
                                E[:], C[:, :, 0:WE2].unsqueeze(3)
                                .broadcast_to((R2, 4, WE2, 2)),
                                ACT.Copy, bias=0.0, scale=1.0)
                            Ef = E[:].rearrange("p c w two -> p c (w two)")
                            my = myp.tile([R2, Wn], bf16, tag="my",
                                          name=f"my{tag}_{dy}")
                            nc.scalar.activation(
                                uy[:], mvy[:], ACT.Abs,
                                bias=bias_ap(-dy, R2), scale=1.0)
                            nc.scalar.activation(
                                my[:], uy[:], ACT.Relu,
                                bias=rvt[:, 0:1], scale=-1.0)
                            myb = my[:].unsqueeze(1).broadcast_to((R2, 4, Wn))
                            for i, kc in enumerate(kcs):
                                mb = (mxk[kc][:].unsqueeze(1)
                                      .broadcast_to((R2, 4, Wn)))
                                src = Ef[:, :, 2 * kc:2 * kc + Wn]
                                if i == 0:
                                    nc.vector.tensor_mul(tmp[:], src, mb)
                                else:
                                    nc.vector.tensor_mul(tm2[:], src, mb)
                                    nc.vector.tensor_add(tmp[:], tmp[:],
                                                         tm2[:])
                            if first:
                                nc.vector.tensor_mul(acc[:], tmp[:], myb)
                                first = False
                            else:
                                nc.vector.tensor_mul(tm2[:], tmp[:], myb)
                                nc.vector.tensor_add(acc[:], acc[:], tm2[:])
                        nc.sync.dma_start(
                            st[j][0:4, r0p:r0p + 2 * R2:2,
                                  stL[j]:stL[j] + Wn]
                            .rearrange("c r w -> r c w"), acc[:])

            # ---------------- conv stage ----------------
            with tc.tile_pool(name="cw", bufs=1) as wpool:
                w1t = [wpool.tile([60, 32], bf16, tag=f"w1t{k}",
                                  name=f"w1t{k}") for k in range(3)]
                w2t = [wpool.tile([96, 32], bf16, tag=f"w2t{k}",
                                  name=f"w2t{k}") for k in range(3)]
                w3t = [wpool.tile([96, 4], bf16, tag=f"w3t{k}",
                                  name=f"w3t{k}") for k in range(3)]
                b1t = wpool.tile([32, 1], f32, tag="b1t")
                b2t = wpool.tile([32, 1], f32, tag="b2t")
                b3t = wpool.tile([4, 1], f32, tag="b3t")
                for k in range(3):
                    nc.sync.dma_start(w1t[k][:], w1[k, :, :])
                    nc.sync.dma_start(w2t[k][:], w2[k, :, :])
                    nc.sync.dma_start(w3t[k][:], w3[k, :, :])
                nc.sync.dma_start(b1t[:], b1[:])
                nc.sync.dma_start(b2t[:], b2[:])
                nc.sync.dma_start(b3t[:], b3[:])
                HW2 = Wn // 2
                for half in range(2):
                    xlo = half * HW2
                    XC = HW2 + 6
                    C1X = HW2 + 4
                    C2X = HW2 + 2
                    for rc0 in range(0, BAND, RCH):
                        orow = s0 + rc0
                        with tc.tile_pool(name=f"cv{half}_{rc0}",
                                          bufs=1) as pool, \
                             tc.tile_pool(name=f"cp{half}_{rc0}", bufs=2,
                                          space="PSUM") as pps:
                            R3 = RCH
                            R2c = RCH + 2
                            R1 = RCH + 4
                            RG = RCH + 6
                            rg = pool.tile([60, RG, XC], bf16, tag="rg")
                            for ky in range(3):
                                rr0 = orow - 3 + ky
                                nc.sync.dma_start(
                                    rg[ky * 20:ky * 20 + 4, :, :],
                                    zu0[:, rr0:rr0 + RG,
                                        xlo:xlo + XC])
                                nc.sync.dma_start(
                                    rg[ky * 20 + 4:ky * 20 + 20, :, :],
                                    st[0][:, rr0:rr0 + RG,
                                          xlo:xlo + XC])
                            rv1 = pool.tile([32, R1], bf16, tag="rv1")
                            rv2 = pool.tile([32, R2c], bf16, tag="rv2")
                            nc.gpsimd.dma_start(
                                rv1[:], rv[orow - 2:orow - 2 + R1, 0:1]
                                .rearrange("r one -> one r")
                                .broadcast_to((32, R1)))
                            nc.gpsimd.dma_start(
                                rv2[:], rv[orow - 1:orow - 1 + R2c, 0:1]
                                .rearrange("r one -> one r")
                                .broadcast_to((32, R2c)))
                            c1 = pool.tile([96, R1, C1X], bf16, tag="c1")
                            for r in range(R1):
                                ps = pps.tile([32, C1X], mybir.dt.float32,
                                              tag="ps1")
                                for kx in range(3):
                                    nc.tensor.matmul(ps[:], w1t[kx][:],
                                                     rg[:, r, kx:kx + C1X],
                                                     start=(kx == 0),
                                                     stop=(kx == 2))
                                nc.scalar.activation(c1[0:32, r, :], ps[:],
                                                     ACT.Relu,
                                                     bias=b1t[:, 0:1],
                                                     scale=1.0)
                            c1v = c1[0:32, :, :]
                            nc.vector.tensor_mul(
                                c1v, c1v,
                                rv1[:].unsqueeze(2)
                                .broadcast_to((32, R1, C1X)))
                            if half == 0:
                                nc.vector.memset(c1[0:32, :, 0:2], 0.0)
                            else:
                                nc.vector.memset(c1[0:32, :, C1X - 2:C1X],
                                                 0.0)
                            nc.sync.dma_start(
                                c1[32:64, 0:R1 - 1, :]
                                .rearrange("p r x -> p (r x)"),
                                c1[0:32, 1:R1, :]
                                .rearrange("p r x -> p (r x)"))
                            nc.sync.dma_start(
                                c1[64:96, 0:R1 - 2, :]
                                .rearrange("p r x -> p (r x)"),
                                c1[0:32, 2:R1, :]
                                .rearrange("p r x -> p (r x)"))
                            c2 = pool.tile([96, R2c, C2X], bf16, tag="c2")
                            for r in range(R2c):
                                ps = pps.tile([32, C2X], mybir.dt.float32,
                                              tag="ps2")
                                for kx in range(3):
                                    nc.tensor.matmul(ps[:], w2t[kx][:],
                                                     c1[:, r, kx:kx + C2X],
                                                     start=(kx == 0),
                                                     stop=(kx == 2))
                                nc.scalar.activation(c2[0:32, r, :], ps[:],
                                                     ACT.Relu,
                                                     bias=b2t[:, 0:1],
                                                     scale=1.0)
                            c2v = c2[0:32, :, :]
                            nc.vector.tensor_mul(
                                c2v, c2v,
                                rv2[:].unsqueeze(2)
                                .broadcast_to((32, R2c, C2X)))
                            if half == 0:
                                nc.vector.memset(c2[0:32, :, 0:1], 0.0)
                            else:
                                nc.vector.memset(c2[0:32, :, C2X - 1:C2X],
                                                 0.0)
                            nc.sync.dma_start(
                                c2[32:64, 0:R2c - 1, :]
                                .rearrange("p r x -> p (r x)"),
                                c2[0:32, 1:R2c, :]
                                .rearrange("p r x -> p (r x)"))
                            nc.sync.dma_start(
                                c2[64:96, 0:R2c - 2, :]
                                .rearrange("p r x -> p (r x)"),
                                c2[0:32, 2:R2c, :]
                                .rearrange("p r x -> p (r x)"))
                            c3 = pool.tile([4, R3, HW2], mybir.dt.float32,
                                           tag="c3")
                            for r in range(R3):
                                ps = pps.tile([4, HW2], mybir.dt.float32,
                                              tag="ps3")
                                for kx in range(3):
                                    nc.tensor.matmul(ps[:], w3t[kx][:],
                                                     c2[:, r, kx:kx + HW2],
                                                     start=(kx == 0),
                                                     stop=(kx == 2))
                                nc.scalar.activation(c3[:, r, :], ps[:],
                                                     ACT.Tanh,
                                                     bias=b3t[:, 0:1],
                                                     scale=1.0)
                            c3f = c3[:].rearrange("p r x -> p (r x)")
                            nc.vector.tensor_scalar_mul(c3f, c3f, 10.0)
                            nc.vector.tensor_scalar_add(c3f, c3f, 10.0)
                            nc.sync.dma_start(
                                out[:, rc0:rc0 + RCH, xlo:xlo + HW2], c3[:])
    nc.compile()
    return nc


# ---------------------------------------------------------------------------
# host prep
# ---------------------------------------------------------------------------

def _host_prep(inputs, plan):
    import ml_dtypes
    bf = ml_dtypes.bfloat16
    color = np.asarray(inputs["color"], np.float32)
    motion = np.asarray(inputs["motion"], np.float32)
    depth = np.asarray(inputs["depth"], np.float32)
    RLoff = plan["RLoff"]

    def rw(wname, inc, outc):
        w_ = np.asarray(inputs[wname], np.float32)
        arr = np.zeros((3, 3 * inc, outc), np.float32)
        for kx in range(3):
            for ky in range(3):
                arr[kx, ky * inc:(ky + 1) * inc, :] = w_[:, :, ky, kx].T
        return arr.astype(bf)

    w1h = rw("rw_w1", 20, 32)
    w2h = rw("rw_w2", 32, 32)
    w3h = rw("rw_w3", 32, 4)
    b1h = np.asarray(inputs["rw_b1"], np.float32).reshape(32, 1)
    b2h = np.asarray(inputs["rw_b2"], np.float32).reshape(32, 1)
    b3h = np.asarray(inputs["rw_b3"], np.float32).reshape(4, 1)

    x4 = np.concatenate([color, depth], axis=2)  # [B,5,4,H,W]

    bias_vals = plan["bias_vals"]
    dych = np.tile(np.asarray(bias_vals, np.float32)[None, :], (128, 1))
    parh = (1.0 - (np.arange(Wn) % 2)).astype(np.float32).reshape(1, Wn)

    in_maps = []
    for core in range(NCORES):
        b = core // NBAND
        g = core % NBAND
        G0 = g * BAND
        RL = G0 + RLoff  # even

        # f0 zero-stuffed rgbd with CM-wide zero x-margins
        zu0h = np.zeros((4, NR, Wn + 2 * CM), np.float32)
        rloc = np.arange(NR)
        A = RL + rloc
        sel = (A >= 0) & (A < Hn) & (A % 2 == 1)
        rs = rloc[sel]
        ois = (A[sel] - 1) // 2
        zu0h[:, rs, CM + 1:CM + Wn:2] = x4[b, 0][:, ois, :]
        zu0h = zu0h.astype(bf)

        # compact per-frame sources: zc[f][c, rc, q] = x4[b,f,c, RL//2+rc, q+KLO]
        zchs = {}
        for f in range(1, 5):
            stg = plan["stages"][f - 1]
            KLO, KC = stg["KLO"], stg["KC"]
            WCp = W - KLO + KC + 2
            zch = np.zeros((4, NRC, WCp), np.float32)
            rcs = np.arange(NRC)
            srow = RL // 2 + rcs
            vr = (srow >= 0) & (srow < H)
            # columns: q maps to small col q + KLO
            q = np.arange(WCp)
            scol = q + KLO
            vc = (scol >= 0) & (scol < W)
            rr = np.clip(srow, 0, H - 1)
            cc = np.clip(scol, 0, W - 1)
            blk = x4[b, f][:, rr, :][:, :, cc]
            blk = blk * vr[None, :, None] * vc[None, None, :]
            zch[:] = blk
            zchs[f"zc{f}"] = zch.astype(bf)

        OB = RL // 2 - 1
        rows = np.clip(OB + np.arange(MR), 0, H - 1)
        cols = np.clip(np.arange(MW) - 1, 0, W - 1)
        moth = np.ascontiguousarray(
            motion[b, 0:4][:, :, rows][:, :, :, cols].astype(np.float32))

        rvh = np.zeros((NR, 1), np.float32)
        rvh[(A >= 0) & (A < Hn), 0] = 1.0

        m = dict(zu0=zu0h, mot=moth, rv=rvh, dyc=dych, par=parh,
                 w1=w1h, w2=w2h, w3=w3h, b1=b1h, b2=b2h, b3=b3h)
        m.update(zchs)
        in_maps.append(m)
    return in_maps


_CACHE = {}
_last_nc = None


def kernel(**inputs):
    global _last_nc
    from concourse.bass_utils import run_bass_kernel_spmd

    motion = np.asarray(inputs["motion"], np.float32)
    plan = _plan(motion)
    key = _plan_key(plan)
    if key not in _CACHE:
        _CACHE[key] = _build(plan)
    nc = _CACHE[key]
    _last_nc = nc

    in_maps = _host_prep(inputs, plan)
    res = run_bass_kernel_spmd(nc, in_maps, core_ids=list(range(NCORES)))
    if res.exec_time_ns is not None:
        print(f"HW exec time: {res.exec_time_ns} ns")
    outf = np.zeros((B, 4, Hn, Wn), np.float32)
    for core in range(NCORES):
        b = core // NBAND
        g = core % NBAND
        outf[b, :, g * BAND:(g + 1) * BAND, :] = res.results[core]["out"]
    return outf
